# revision 10
# baseline (speedup 1.0000x reference)
"""Causal self-attention (softmax over the QUERY axis) for Trainium2, 8 cores.

Reference semantics (B=2, S=2048, D=1024, H=16, HD=64):
    q = x @ Wq; k = x @ Wk; v = x @ Wv          (per batch)
    s[b,h,q,k] = <q_bqh, k_bkh>;  mask k > q -> -inf
    w = softmax(s / sqrt(1024), axis=q)          # normalize over QUERY axis
    ctx[b,q,h,:] = sum_k w[b,h,q,k] * v[b,k,h,:]

Sharding: core c handles batch b = c // 4 and head group g = c % 4
(4 heads: 4g..4g+3).  Per core everything is done in a transposed
score layout S^T[k, q], which makes the query-axis softmax a FREE-AXIS
reduction, and the 1/Z[k] normalizer folds into V rows (no per-element
divide): ctx[q,d] = sum_k exp(s)/Z[k] * v[k,d] = sum_k exp(s) * (v[k,d]/Z[k]).

Key structure (v2, ACT-engine-centric):
  - Causal diag masking is done IN PSUM via one extra matmul per score
    row: I128^T @ TRI adds -1e6 to the strictly-lower part of the 128x128
    diagonal block, so exp() produces exact zeros and the row sum (Z) is
    correct with no post-hoc correction (no gpsimd selects, no inv sums).
  - Score rows ping-pong between a 4-bank [128,2048] and a 3-bank
    [128,1536] PSUM tile, so each row is ONE activation instruction;
    short rows (kt>=8) are packed in pairs into one activation.
  - Z: accum_out on solo rows (kt 0..7), DVE post-zero row-reduce for
    packed rows (kt 8..15).
  - exp() is the only real work on the Scalar queue (input DMAs moved to
    gpsimd/vector queues); E is stored per head as one packed [128,17408]
    bf16 tile (row kt at col E_OFF[kt]).
  - A short burst of dummy matmuls during the input-DMA window pre-warms
    the PE HAM clock gate so real matmuls start at 2.4 GHz.

Device layouts (per core):
    xT  [1024, 2048] bf16 (host-transposed)  -> SBUF [128, 8, 2048]
    Wq/Wk/Wv column slices [1024, 256] bf16  -> SBUF [128, 8, 256]
    qT/kT  [128(2 heads x 64), 2 pairs, 2048] bf16 (projection output)
    v      [128(s in tile), 16 kt, 256(4 heads x 64)] bf16 (scaled by 1/Z in place)
    E      per head [128, 17408] bf16, row kt at cols [E_OFF[kt], +2048-128kt)
    out    [256(4 heads x 64), 2048] f32 = ctx^T; host transposes back.
"""

import numpy as np
import ml_dtypes
from contextlib import ExitStack

import concourse.bass as bass
import concourse.tile as tile
from concourse import bacc, mybir
from concourse.bass_utils import run_bass_kernel_spmd

BF16 = mybir.dt.bfloat16
F32 = mybir.dt.float32

B, S, D, H, HD = 2, 2048, 1024, 16, 64
NCORES = 8
HL = 4                       # heads per core
KC = D // 128                # 8 contraction chunks
KT = S // 128                # 16 key tiles
QC = S // 512                # 4 query chunks of 512
SCALE = 1.0 / float(np.sqrt(np.float32(D)))   # 1/32
MASK_BIG = -1.0e6

W_ROW = [S - 128 * kt for kt in range(KT)]          # valid width of E row kt
E_OFF = np.concatenate([[0], np.cumsum(W_ROW)]).astype(int)
E_TOT = int(E_OFF[-1])                              # 17408

# score-row emission plan per query chunk (descending heads 0..2):
#   each entry: (rows_tuple, which_psum_tile)  'A' = [128,2048], 'B' = [128,1536]
QC_GROUPS = {
    3: [((14, 15), 'B'), ((12, 13), 'A')],
    2: [((10, 11), 'B'), ((8, 9), 'A')],
    1: [((7,), 'B'), ((6,), 'A'), ((5,), 'B'), ((4,), 'A')],
    0: [((3,), 'A'), ((2,), 'A'), ((1,), 'A'), ((0,), 'A')],
}
# head 3 runs ascending in groups of 4 rows so pair-1 ctx can start early
G_GROUPS = {
    0: [((0,), 'A'), ((1,), 'A'), ((2,), 'A'), ((3,), 'A')],
    1: [((4,), 'B'), ((5,), 'A'), ((6,), 'B'), ((7,), 'A')],
    2: [((8, 9), 'A'), ((10, 11), 'B')],
    3: [((12, 13), 'A'), ((14, 15), 'B')],
}


def _emit(ctx: ExitStack, tc: tile.TileContext, out_ap, xT, wq, wk, wv, ident, tri):
    nc = tc.nc
    Exp = mybir.ActivationFunctionType.Exp

    consts = ctx.enter_context(tc.tile_pool(name="consts", bufs=1))
    qkp = ctx.enter_context(tc.tile_pool(name="qk", bufs=1))
    vp = ctx.enter_context(tc.tile_pool(name="v", bufs=1))
    epool = ctx.enter_context(tc.tile_pool(name="e", bufs=3))
    zpool = ctx.enter_context(tc.tile_pool(name="z", bufs=4))
    outp = ctx.enter_context(tc.tile_pool(name="outp", bufs=1))
    # PSUM: 4-bank + 3-bank score tiles (ping-pong) + one bank for proj/ctx
    psA = ctx.enter_context(tc.tile_pool(name="psA", bufs=1, space="PSUM"))
    psB = ctx.enter_context(tc.tile_pool(name="psB", bufs=1, space="PSUM"))
    small_ps = ctx.enter_context(tc.tile_pool(name="small_ps", bufs=1, space="PSUM"))

    # ---- input DMAs: both HWDGE rings (sync + scalar).  The two scalar-ring
    # issues happen before any activation work exists, so the ACT queue is
    # free again well before the first exp ----
    xT_r = xT.rearrange("(c p) s -> p c s", p=128)
    xT_cs = [None] * 4

    def load_chunk(sc, eng):
        xT_cs[sc] = consts.tile([128, KC, 512], BF16, tag=f"xT{sc}",
                                name=f"xT{sc}_sb")
        eng.dma_start(out=xT_cs[sc], in_=xT_r[:, :, 512 * sc:512 * sc + 512])

    w_sb = {}

    def load_w(name, t):
        w_sb[name] = consts.tile([128, KC, HL * HD], BF16, tag=f"w{name}",
                                 name=f"w{name}_sb")
        nc.sync.dma_start(out=w_sb[name], in_=t.rearrange("(c p) n -> p c n", p=128))

    load_w("q", wq)
    load_w("k", wk)
    load_w("v", wv)
    for sc in (3, 2, 1, 0):
        load_chunk(sc, nc.scalar)

    # mask constants built on-device (gpsimd) -- no DMA descriptors needed.
    # ident = keep j==p of ones (two is_ge selects); tri = -1e6 where j < p.
    ident_sb = consts.tile([128, 128], BF16, tag="ident", name="ident_sb")
    tri_sb = consts.tile([128, 128], BF16, tag="tri", name="tri_sb")
    scr_sb = consts.tile([128, 128], BF16, tag="scr", name="scr_sb")
    nc.gpsimd.memset(scr_sb, 1.0)
    nc.gpsimd.affine_select(ident_sb, scr_sb, pattern=[[1, 128]],
                            compare_op=mybir.AluOpType.is_ge, fill=0.0,
                            base=0, channel_multiplier=-1)
    nc.gpsimd.affine_select(ident_sb, ident_sb, pattern=[[-1, 128]],
                            compare_op=mybir.AluOpType.is_ge, fill=0.0,
                            base=0, channel_multiplier=1)
    nc.gpsimd.memset(scr_sb, MASK_BIG)
    nc.gpsimd.affine_select(tri_sb, scr_sb, pattern=[[-1, 128]],
                            compare_op=mybir.AluOpType.is_ge, fill=0.0,
                            base=-1, channel_multiplier=1)

    def xT_slice(c, lo, w):
        sc, o = divmod(lo, 512)
        assert o + w <= 512
        return xT_cs[sc][:, c, o:o + w]

    qT_sb = qkp.tile([128, 2, S], BF16, tag="qT")
    kT_sb = qkp.tile([128, 2, S], BF16, tag="kT")
    v_sb = vp.tile([128, KT, HL * HD], BF16, tag="v")
    out_sb = outp.tile([128, 2, S], F32, tag="out")
    sp_tile = small_ps.tile([128, 512], F32, tag="ps", name="sp")
    sp_par = [0]

    def next_sp():
        s = sp_tile[:, 256 * sp_par[0]:256 * sp_par[0] + 256]
        sp_par[0] ^= 1
        return s

    # ---- PE warm-up: dummy matmuls during the DMA window so HAM reaches
    # K=8/8 before the first projection chain (8 disjoint regions so no
    # write-after-write sync gets inserted between them) ----
    warm = consts.tile([128, 256], BF16, tag="warm", name="warm_sb")
    nc.vector.memset(warm, 0.0)
    wps = psA.tile([128, 2048], F32, tag="sA", name="warmps")
    for i in range(30):
        r = 256 * (i % 8)
        nc.tensor.matmul(wps[:, r:r + 256], warm[:, 0:128], warm,
                         start=True, stop=True)

    def proj_chain(name, pair, qc, half):
        dst = qT_sb if name == "q" else kT_sb
        ps = next_sp()
        for c in range(KC):
            nc.tensor.matmul(
                ps,
                w_sb[name][:, c, 128 * pair:128 * pair + 128],
                xT_cs[qc][:, c, 256 * half:256 * half + 256],
                start=(c == 0), stop=(c == KC - 1),
            )
        lo = 512 * qc + 256 * half
        nc.vector.tensor_copy(dst[:, pair, lo:lo + 256], ps)

    def proj_v():
        # v natural layout: out partitions = s-within-tile, cols = 4 heads x 64
        for st in range(KT):
            ps = next_sp()
            for c in range(KC):
                nc.tensor.matmul(
                    ps,
                    xT_slice(c, 128 * st, 128),
                    w_sb["v"][:, c, :],
                    start=(c == 0), stop=(c == KC - 1),
                )
            nc.vector.tensor_copy(v_sb[:, st, :], ps)

    def alloc_head(h):
        return {
            "h": h,
            "E": epool.tile([128, E_TOT], BF16, tag="E", name=f"E{h}"),
            "zp": zpool.tile([128, KT], F32, tag="zp", name=f"zp{h}"),
            "zi": zpool.tile([128, KT], F32, tag="zi", name=f"zi{h}"),
        }

    def score_group(st, rows, which):
        """Matmul rows into one PSUM tile (with in-PSUM causal mask via the
        I^T @ TRI trick), one exp() activation, then per-row Z."""
        h = st["h"]
        pair, half = divmod(h, 2)
        pb = 64 * half
        offs = []
        o = 0
        for kt in rows:
            offs.append(o)
            o += W_ROW[kt]
        wtot = o
        if which == 'A':
            ps = psA.tile([128, 2048], F32, tag="sA", name="sA")
        else:
            ps = psB.tile([128, 1536], F32, tag="sB", name="sB")
        for kt, po in zip(rows, offs):
            q0k = 128 * kt
            W = W_ROW[kt]
            lhsT = kT_sb[pb:pb + 64, pair, q0k:q0k + 128]   # [64, 128]
            # chunks split at PSUM 512 (bank) boundaries
            c = po
            first = True
            while c < po + W:
                c1 = min(po + W, (c // 512 + 1) * 512)
                nc.tensor.matmul(
                    ps[:, c:c1],
                    lhsT,
                    qT_sb[pb:pb + 64, pair, q0k + c - po:q0k + c1 - po],
                    start=True, stop=not first,
                )
                first = False
                c = c1
            # causal mask: add -1e6 on the strictly-lower part of the
            # 128x128 diagonal block (q < k) so exp() yields exact zeros
            nc.tensor.matmul(
                ps[:, po:po + 128], ident_sb, tri_sb,
                start=False, stop=True,
            )
        e_dst = st["E"][:, int(E_OFF[rows[0]]):int(E_OFF[rows[0]]) + wtot]
        if len(rows) == 1:
            kt = rows[0]
            nc.scalar.activation(e_dst, ps[:, 0:wtot], Exp, scale=SCALE,
                                 accum_out=st["zp"][:, kt:kt + 1])
        else:
            nc.scalar.activation(e_dst, ps[:, 0:wtot], Exp, scale=SCALE)
            for kt in rows:
                nc.vector.tensor_reduce(
                    st["zp"][:, kt:kt + 1],
                    st["E"][:, int(E_OFF[kt]):int(E_OFF[kt]) + W_ROW[kt]],
                    axis=mybir.AxisListType.X, op=mybir.AluOpType.add,
                )

    def v2_scale(st, k0, k1):
        """finalize 1/Z for rows [k0, k1) and scale this head's V cols."""
        h = st["h"]
        nc.vector.reciprocal(st["zi"][:, k0:k1], st["zp"][:, k0:k1])
        zia = st["zi"][:, k0:k1]
        zi_bc = bass.AP(tensor=zia.tensor, offset=zia.offset,
                        ap=[zia.ap[0], zia.ap[1], [0, HD]])
        nc.vector.tensor_mul(
            v_sb[:, k0:k1, HD * h:HD * h + HD],
            v_sb[:, k0:k1, HD * h:HD * h + HD],
            zi_bc,
        )

    def ctx_pair(sta, stb, qc):
        """col-packed ctx chains for a whole pair (heads sta, stb) at qc,
        in two 256-col halves so the psum tiles double-buffer in 1 bank."""
        pair = sta["h"] // 2
        for h256 in (0, 1):
            lo_q = 512 * qc + 256 * h256
            ps = next_sp()
            n_kt = 4 * qc + 2 * h256 + 2
            for kt in range(n_kt):
                q0 = max(lo_q, 128 * kt)
                w = lo_q + 256 - q0
                for half, st in ((0, sta), (1, stb)):
                    h = st["h"]
                    lo = int(E_OFF[kt]) + q0 - 128 * kt
                    nc.tensor.matmul(
                        ps[64 * half:64 * half + 64, q0 - lo_q:256],
                        v_sb[:, kt, HD * h:HD * h + HD],
                        st["E"][:, lo:lo + w],
                        start=(kt == 0), stop=(kt == n_kt - 1),
                        tile_position=(0, 64 * half),
                        skip_group_check=True,
                    )
            nc.vector.tensor_copy(out_sb[:, pair, lo_q:lo_q + 256], ps)

    def out_dma(pair, qc):
        nc.sync.dma_start(
            out=out_ap[128 * pair:128 * pair + 128, 512 * qc:512 * qc + 512],
            in_=out_sb[:, pair, 512 * qc:512 * qc + 512],
        )

    # ---- emission (order = scheduling priority) ----
    st0 = alloc_head(0)
    for qc in (3, 2, 1, 0):           # head 0 interleaved with its projections
        for half in (0, 1):
            proj_chain("q", 0, qc, half)
        for half in (0, 1):
            proj_chain("k", 0, qc, half)
        for rows, which in QC_GROUPS[qc]:
            score_group(st0, rows, which)
    st1 = alloc_head(1)
    for qc in (3, 2, 1, 0):           # head 1: projections already done
        for rows, which in QC_GROUPS[qc]:
            score_group(st1, rows, which)
    for qc in (3, 2, 1, 0):           # pair-1 projections: filler under
        for half in (0, 1):           # heads 0-1's exp backlog
            proj_chain("q", 1, qc, half)
        for half in (0, 1):
            proj_chain("k", 1, qc, half)
    st2 = alloc_head(2)
    for qc in (3, 2, 1, 0):           # head 2 rows follow head 1 immediately
        for rows, which in QC_GROUPS[qc]:
            score_group(st2, rows, which)
    proj_v()                          # fillers under head-2's exp backlog
    v2_scale(st0, 0, KT)
    v2_scale(st1, 0, KT)
    for g in range(4):
        ctx_pair(st0, st1, g)
        out_dma(0, g)
    v2_scale(st2, 0, KT)
    st3 = alloc_head(3)
    for g in range(4):                # head 3 ascending; pair-1 ctx follows
        for rows, which in G_GROUPS[g]:
            score_group(st3, rows, which)
        v2_scale(st3, 4 * g, 4 * g + 4)
        ctx_pair(st2, st3, g)
        out_dma(1, g)


_PROG = None


def _build_program():
    global _PROG
    if _PROG is not None:
        return _PROG
    nc = bacc.Bacc("TRN2", target_bir_lowering=False, debug=False,
                   num_devices=NCORES)
    xT = nc.dram_tensor("xT", [D, S], BF16, kind="ExternalInput").ap()
    wq = nc.dram_tensor("wq", [D, HL * HD], BF16, kind="ExternalInput").ap()
    wk = nc.dram_tensor("wk", [D, HL * HD], BF16, kind="ExternalInput").ap()
    wv = nc.dram_tensor("wv", [D, HL * HD], BF16, kind="ExternalInput").ap()
    ident = nc.dram_tensor("ident", [128, 128], BF16, kind="ExternalInput").ap()
    tri = nc.dram_tensor("tri", [128, 128], BF16, kind="ExternalInput").ap()
    out = nc.dram_tensor("out", [HL * HD, S], F32, kind="ExternalOutput").ap()
    with tile.TileContext(nc) as tc:
        with ExitStack() as stack:
            _emit(stack, tc, out, xT, wq, wk, wv, ident, tri)
    nc.compile()
    _PROG = nc
    return nc


def make_in_maps(x, Wq, Wk, Wv):
    bf = ml_dtypes.bfloat16
    ident = np.eye(128, dtype=bf)
    tri = np.tril(np.full((128, 128), MASK_BIG, np.float32), -1).astype(bf)
    in_maps = []
    for core in range(NCORES):
        b, g = divmod(core, NCORES // B)
        cols = slice(HL * HD * g, HL * HD * (g + 1))
        in_maps.append({
            "xT": np.ascontiguousarray(np.asarray(x[b]).T).astype(bf),
            "wq": np.ascontiguousarray(np.asarray(Wq)[:, cols]).astype(bf),
            "wk": np.ascontiguousarray(np.asarray(Wk)[:, cols]).astype(bf),
            "wv": np.ascontiguousarray(np.asarray(Wv)[:, cols]).astype(bf),
            "ident": ident,
            "tri": tri,
        })
    return in_maps


def assemble(results):
    out = np.empty((B, S, H * HD), np.float32)
    for core in range(NCORES):
        b, g = divmod(core, NCORES // B)
        out[b, :, HL * HD * g:HL * HD * (g + 1)] = results[core]["out"].T
    return out


def kernel(**inputs):
    nc = _build_program()
    in_maps = make_in_maps(inputs["x"], inputs["Wq"], inputs["Wk"], inputs["Wv"])
    res = run_bass_kernel_spmd(nc, in_maps, list(range(NCORES)))
    return assemble(res.results)


# revision 12
# speedup vs baseline: 1.0428x; 1.0428x over previous
"""Causal self-attention (softmax over the QUERY axis) for Trainium2, 8 cores.

Reference semantics (B=2, S=2048, D=1024, H=16, HD=64):
    q = x @ Wq; k = x @ Wk; v = x @ Wv          (per batch)
    s[b,h,q,k] = <q_bqh, k_bkh>;  mask k > q -> -inf
    w = softmax(s / sqrt(1024), axis=q)          # normalize over QUERY axis
    ctx[b,q,h,:] = sum_k w[b,h,q,k] * v[b,k,h,:]

Sharding: core c handles batch b = c // 4 and head group g = c % 4
(4 heads: 4g..4g+3).  Per core everything is done in a transposed
score layout S^T[k, q], which makes the query-axis softmax a FREE-AXIS
reduction, and the 1/Z[k] normalizer folds into V rows (no per-element
divide): ctx[q,d] = sum_k exp(s)/Z[k] * v[k,d] = sum_k exp(s) * (v[k,d]/Z[k]).

Key structure (v2, ACT-engine-centric):
  - Causal diag masking is done IN PSUM via one extra matmul per score
    row: I128^T @ TRI adds -1e6 to the strictly-lower part of the 128x128
    diagonal block, so exp() produces exact zeros and the row sum (Z) is
    correct with no post-hoc correction (no gpsimd selects, no inv sums).
  - Score rows ping-pong between a 4-bank [128,2048] and a 3-bank
    [128,1536] PSUM tile, so each row is ONE activation instruction;
    short rows (kt>=8) are packed in pairs into one activation.
  - Z: accum_out on solo rows (kt 0..7), DVE post-zero row-reduce for
    packed rows (kt 8..15).
  - exp() is the only real work on the Scalar queue (input DMAs moved to
    gpsimd/vector queues); E is stored per head as one packed [128,17408]
    bf16 tile (row kt at col E_OFF[kt]).
  - A short burst of dummy matmuls during the input-DMA window pre-warms
    the PE HAM clock gate so real matmuls start at 2.4 GHz.

Device layouts (per core):
    xT  [1024, 2048] bf16 (host-transposed)  -> SBUF [128, 8, 2048]
    Wq/Wk/Wv column slices [1024, 256] bf16  -> SBUF [128, 8, 256]
    qT/kT  [128(2 heads x 64), 2 pairs, 2048] bf16 (projection output)
    v      [128(s in tile), 16 kt, 256(4 heads x 64)] bf16 (scaled by 1/Z in place)
    E      per head [128, 17408] bf16, row kt at cols [E_OFF[kt], +2048-128kt)
    out    [256(4 heads x 64), 2048] f32 = ctx^T; host transposes back.
"""

import numpy as np
import ml_dtypes
from contextlib import ExitStack

import concourse.bass as bass
import concourse.tile as tile
from concourse import bacc, mybir
from concourse.bass_utils import run_bass_kernel_spmd

BF16 = mybir.dt.bfloat16
F32 = mybir.dt.float32

B, S, D, H, HD = 2, 2048, 1024, 16, 64
NCORES = 8
HL = 4                       # heads per core
KC = D // 128                # 8 contraction chunks
KT = S // 128                # 16 key tiles
QC = S // 512                # 4 query chunks of 512
SCALE = 1.0 / float(np.sqrt(np.float32(D)))   # 1/32
MASK_BIG = -1.0e6

W_ROW = [S - 128 * kt for kt in range(KT)]          # valid width of E row kt
E_OFF = np.concatenate([[0], np.cumsum(W_ROW)]).astype(int)
E_TOT = int(E_OFF[-1])                              # 17408

# score-row emission plan per query chunk (descending heads 0..2):
#   each entry: (rows_tuple, which_psum_tile)  'A' = [128,2048], 'B' = [128,1536]
QC_GROUPS = {
    3: [((14, 15), 'B'), ((12, 13), 'A')],
    2: [((10, 11), 'B'), ((8, 9), 'A')],
    1: [((7,), 'B'), ((6,), 'A'), ((5,), 'B'), ((4,), 'A')],
    0: [((3,), 'A'), ((2,), 'A'), ((1,), 'A'), ((0,), 'A')],
}
# head 3 runs ascending in groups of 4 rows so pair-1 ctx can start early
G_GROUPS = {
    0: [((0,), 'A'), ((1,), 'A'), ((2,), 'A'), ((3,), 'A')],
    1: [((4,), 'B'), ((5,), 'A'), ((6,), 'B'), ((7,), 'A')],
    2: [((8, 9), 'A'), ((10, 11), 'B')],
    3: [((12, 13), 'A'), ((14, 15), 'B')],
}


def _emit(ctx: ExitStack, tc: tile.TileContext, out_ap, xT, wq, wk, wv):
    nc = tc.nc
    Exp = mybir.ActivationFunctionType.Exp

    consts = ctx.enter_context(tc.tile_pool(name="consts", bufs=1))
    qkp = ctx.enter_context(tc.tile_pool(name="qk", bufs=1))
    vp = ctx.enter_context(tc.tile_pool(name="v", bufs=1))
    epool = ctx.enter_context(tc.tile_pool(name="e", bufs=3))
    zpool = ctx.enter_context(tc.tile_pool(name="z", bufs=4))
    outp = ctx.enter_context(tc.tile_pool(name="outp", bufs=1))
    # PSUM: 4-bank + 3-bank score tiles (ping-pong) + one bank for proj/ctx
    psA = ctx.enter_context(tc.tile_pool(name="psA", bufs=1, space="PSUM"))
    psB = ctx.enter_context(tc.tile_pool(name="psB", bufs=1, space="PSUM"))
    small_ps = ctx.enter_context(tc.tile_pool(name="small_ps", bufs=1, space="PSUM"))

    # ---- input DMAs: both HWDGE rings (sync + scalar).  The two scalar-ring
    # issues happen before any activation work exists, so the ACT queue is
    # free again well before the first exp ----
    xT_r = xT.rearrange("(c p) s -> p c s", p=128)
    xT_cs = [None] * 4

    def load_chunk(sc, eng):
        xT_cs[sc] = consts.tile([128, KC, 512], BF16, tag=f"xT{sc}",
                                name=f"xT{sc}_sb")
        eng.dma_start(out=xT_cs[sc], in_=xT_r[:, :, 512 * sc:512 * sc + 512])

    w_sb = {}

    def load_w(name, t):
        w_sb[name] = consts.tile([128, KC, HL * HD], BF16, tag=f"w{name}",
                                 name=f"w{name}_sb")
        nc.sync.dma_start(out=w_sb[name], in_=t.rearrange("(c p) n -> p c n", p=128))

    load_w("q", wq)
    load_w("k", wk)
    load_w("v", wv)
    for sc in (3, 2, 1, 0):
        load_chunk(sc, nc.scalar)

    # mask constants built on-device (gpsimd) -- no DMA descriptors needed.
    # ident = keep j==p of ones (two is_ge selects); tri = -1e6 where j < p.
    ident_sb = consts.tile([128, 128], BF16, tag="ident", name="ident_sb")
    tri_sb = consts.tile([128, 128], BF16, tag="tri", name="tri_sb")
    scr_sb = consts.tile([128, 128], BF16, tag="scr", name="scr_sb")
    nc.gpsimd.memset(scr_sb, 1.0)
    nc.gpsimd.affine_select(ident_sb, scr_sb, pattern=[[1, 128]],
                            compare_op=mybir.AluOpType.is_ge, fill=0.0,
                            base=0, channel_multiplier=-1)
    nc.gpsimd.affine_select(ident_sb, ident_sb, pattern=[[-1, 128]],
                            compare_op=mybir.AluOpType.is_ge, fill=0.0,
                            base=0, channel_multiplier=1)
    nc.gpsimd.memset(scr_sb, MASK_BIG)
    nc.gpsimd.affine_select(tri_sb, scr_sb, pattern=[[-1, 128]],
                            compare_op=mybir.AluOpType.is_ge, fill=0.0,
                            base=-1, channel_multiplier=1)

    def xT_slice(c, lo, w):
        sc, o = divmod(lo, 512)
        assert o + w <= 512
        return xT_cs[sc][:, c, o:o + w]

    qT_sb = qkp.tile([128, 2, S], BF16, tag="qT")
    kT_sb = qkp.tile([128, 2, S], BF16, tag="kT")
    v_sb = vp.tile([128, KT, HL * HD], BF16, tag="v")
    out_sb = outp.tile([128, 2, S], F32, tag="out")
    sp_tile = small_ps.tile([128, 512], F32, tag="ps", name="sp")
    sp_par = [0]

    def next_sp():
        s = sp_tile[:, 256 * sp_par[0]:256 * sp_par[0] + 256]
        sp_par[0] ^= 1
        return s

    # ---- PE warm-up: dummy matmuls during the DMA window so HAM reaches
    # K=8/8 before the first projection chain (8 disjoint regions so no
    # write-after-write sync gets inserted between them) ----
    warm = consts.tile([128, 256], BF16, tag="warm", name="warm_sb")
    nc.vector.memset(warm, 0.0)
    wps = psA.tile([128, 2048], F32, tag="sA", name="warmps")
    for i in range(30):
        r = 256 * (i % 8)
        nc.tensor.matmul(wps[:, r:r + 256], warm[:, 0:128], warm,
                         start=True, stop=True)

    def proj_chain(name, pair, qc, half):
        dst = qT_sb if name == "q" else kT_sb
        ps = next_sp()
        for c in range(KC):
            nc.tensor.matmul(
                ps,
                w_sb[name][:, c, 128 * pair:128 * pair + 128],
                xT_cs[qc][:, c, 256 * half:256 * half + 256],
                start=(c == 0), stop=(c == KC - 1),
            )
        lo = 512 * qc + 256 * half
        nc.vector.tensor_copy(dst[:, pair, lo:lo + 256], ps)

    def proj_v(s0, s1):
        # v natural layout: out partitions = s-within-tile, cols = 4 heads x 64
        # (chain st only touches xT chunk st//4)
        for st in range(s0, s1):
            ps = next_sp()
            for c in range(KC):
                nc.tensor.matmul(
                    ps,
                    xT_slice(c, 128 * st, 128),
                    w_sb["v"][:, c, :],
                    start=(c == 0), stop=(c == KC - 1),
                )
            nc.vector.tensor_copy(v_sb[:, st, :], ps)

    def alloc_head(h):
        return {
            "h": h,
            "E": epool.tile([128, E_TOT], BF16, tag="E", name=f"E{h}"),
            "zp": zpool.tile([128, KT], F32, tag="zp", name=f"zp{h}"),
            "zi": zpool.tile([128, KT], F32, tag="zi", name=f"zi{h}"),
        }

    def score_group(st, rows, which):
        """Matmul rows into one PSUM tile (with in-PSUM causal mask via the
        I^T @ TRI trick), one exp() activation, then per-row Z."""
        h = st["h"]
        pair, half = divmod(h, 2)
        pb = 64 * half
        offs = []
        o = 0
        for kt in rows:
            offs.append(o)
            o += W_ROW[kt]
        wtot = o
        if which == 'A':
            ps = psA.tile([128, 2048], F32, tag="sA", name="sA")
        else:
            ps = psB.tile([128, 1536], F32, tag="sB", name="sB")
        for kt, po in zip(rows, offs):
            q0k = 128 * kt
            W = W_ROW[kt]
            lhsT = kT_sb[pb:pb + 64, pair, q0k:q0k + 128]   # [64, 128]
            # chunks split at PSUM 512 (bank) boundaries
            c = po
            first = True
            while c < po + W:
                c1 = min(po + W, (c // 512 + 1) * 512)
                nc.tensor.matmul(
                    ps[:, c:c1],
                    lhsT,
                    qT_sb[pb:pb + 64, pair, q0k + c - po:q0k + c1 - po],
                    start=True, stop=not first,
                )
                first = False
                c = c1
            # causal mask: add -1e6 on the strictly-lower part of the
            # 128x128 diagonal block (q < k) so exp() yields exact zeros
            nc.tensor.matmul(
                ps[:, po:po + 128], ident_sb, tri_sb,
                start=False, stop=True,
            )
        e_dst = st["E"][:, int(E_OFF[rows[0]]):int(E_OFF[rows[0]]) + wtot]
        if len(rows) == 1:
            kt = rows[0]
            nc.scalar.activation(e_dst, ps[:, 0:wtot], Exp, scale=SCALE,
                                 accum_out=st["zp"][:, kt:kt + 1])
        else:
            nc.scalar.activation(e_dst, ps[:, 0:wtot], Exp, scale=SCALE)
            for kt in rows:
                nc.vector.tensor_reduce(
                    st["zp"][:, kt:kt + 1],
                    st["E"][:, int(E_OFF[kt]):int(E_OFF[kt]) + W_ROW[kt]],
                    axis=mybir.AxisListType.X, op=mybir.AluOpType.add,
                )

    def v2_scale(st, k0, k1):
        """finalize 1/Z for rows [k0, k1) and scale this head's V cols."""
        h = st["h"]
        nc.vector.reciprocal(st["zi"][:, k0:k1], st["zp"][:, k0:k1])
        zia = st["zi"][:, k0:k1]
        zi_bc = bass.AP(tensor=zia.tensor, offset=zia.offset,
                        ap=[zia.ap[0], zia.ap[1], [0, HD]])
        nc.vector.tensor_mul(
            v_sb[:, k0:k1, HD * h:HD * h + HD],
            v_sb[:, k0:k1, HD * h:HD * h + HD],
            zi_bc,
        )

    def ctx_pair(sta, stb, qc):
        """col-packed ctx chains for a whole pair (heads sta, stb) at qc,
        in two 256-col halves so the psum tiles double-buffer in 1 bank."""
        pair = sta["h"] // 2
        for h256 in (0, 1):
            lo_q = 512 * qc + 256 * h256
            ps = next_sp()
            n_kt = 4 * qc + 2 * h256 + 2
            for kt in range(n_kt):
                q0 = max(lo_q, 128 * kt)
                w = lo_q + 256 - q0
                for half, st in ((0, sta), (1, stb)):
                    h = st["h"]
                    lo = int(E_OFF[kt]) + q0 - 128 * kt
                    nc.tensor.matmul(
                        ps[64 * half:64 * half + 64, q0 - lo_q:256],
                        v_sb[:, kt, HD * h:HD * h + HD],
                        st["E"][:, lo:lo + w],
                        start=(kt == 0), stop=(kt == n_kt - 1),
                        tile_position=(0, 64 * half),
                        skip_group_check=True,
                    )
            nc.vector.tensor_copy(out_sb[:, pair, lo_q:lo_q + 256], ps)

    def out_dma(pair, qc):
        nc.sync.dma_start(
            out=out_ap[128 * pair:128 * pair + 128, 512 * qc:512 * qc + 512],
            in_=out_sb[:, pair, 512 * qc:512 * qc + 512],
        )

    # ---- emission (order = scheduling priority) ----
    # Pair 0 (heads 0,1) is processed per query chunk, zipped, so the exp
    # stream starts as soon as xT chunk 3 lands and never waits on later
    # chunks; pair-1 projections trail each chunk as PE filler.
    st0 = alloc_head(0)
    st1 = alloc_head(1)
    for qc in (3, 2, 1, 0):
        for half in (0, 1):
            proj_chain("q", 0, qc, half)
        for half in (0, 1):
            proj_chain("k", 0, qc, half)
        for rows, which in QC_GROUPS[qc]:
            score_group(st0, rows, which)
        for rows, which in QC_GROUPS[qc]:
            score_group(st1, rows, which)
        for half in (0, 1):
            proj_chain("q", 1, qc, half)
        for half in (0, 1):
            proj_chain("k", 1, qc, half)
    proj_v(0, 4)
    v2_scale(st0, 0, 4)
    v2_scale(st1, 0, 4)
    ctx_pair(st0, st1, 0)
    out_dma(0, 0)
    # Head 2, with the pair-0 context chains and V projection trailing each
    # chunk as PE filler (E0/E1 are fully consumed before head 3 needs the
    # E pool slot).
    st2 = alloc_head(2)
    for qc, (vs0, g) in zip((3, 2, 1, 0), ((4, 1), (8, 2), (12, 3), (None, None))):
        for rows, which in QC_GROUPS[qc]:
            score_group(st2, rows, which)
        if vs0 is not None:
            proj_v(vs0, vs0 + 4)
            v2_scale(st0, vs0, vs0 + 4)
            v2_scale(st1, vs0, vs0 + 4)
            ctx_pair(st0, st1, g)
            out_dma(0, g)
    v2_scale(st2, 0, KT)
    st3 = alloc_head(3)
    for g in range(4):                # head 3 ascending; pair-1 ctx follows
        for rows, which in G_GROUPS[g]:
            score_group(st3, rows, which)
        v2_scale(st3, 4 * g, 4 * g + 4)
        ctx_pair(st2, st3, g)
        out_dma(1, g)


_PROG = None


def _build_program():
    global _PROG
    if _PROG is not None:
        return _PROG
    nc = bacc.Bacc("TRN2", target_bir_lowering=False, debug=False,
                   num_devices=NCORES)
    xT = nc.dram_tensor("xT", [D, S], BF16, kind="ExternalInput").ap()
    wq = nc.dram_tensor("wq", [D, HL * HD], BF16, kind="ExternalInput").ap()
    wk = nc.dram_tensor("wk", [D, HL * HD], BF16, kind="ExternalInput").ap()
    wv = nc.dram_tensor("wv", [D, HL * HD], BF16, kind="ExternalInput").ap()
    out = nc.dram_tensor("out", [HL * HD, S], F32, kind="ExternalOutput").ap()
    with tile.TileContext(nc) as tc:
        with ExitStack() as stack:
            _emit(stack, tc, out, xT, wq, wk, wv)
    nc.compile()
    _PROG = nc
    return nc


def make_in_maps(x, Wq, Wk, Wv):
    bf = ml_dtypes.bfloat16
    in_maps = []
    for core in range(NCORES):
        b, g = divmod(core, NCORES // B)
        cols = slice(HL * HD * g, HL * HD * (g + 1))
        in_maps.append({
            "xT": np.ascontiguousarray(np.asarray(x[b]).T).astype(bf),
            "wq": np.ascontiguousarray(np.asarray(Wq)[:, cols]).astype(bf),
            "wk": np.ascontiguousarray(np.asarray(Wk)[:, cols]).astype(bf),
            "wv": np.ascontiguousarray(np.asarray(Wv)[:, cols]).astype(bf),
        })
    return in_maps


def assemble(results):
    out = np.empty((B, S, H * HD), np.float32)
    for core in range(NCORES):
        b, g = divmod(core, NCORES // B)
        out[b, :, HL * HD * g:HL * HD * (g + 1)] = results[core]["out"].T
    return out


def kernel(**inputs):
    nc = _build_program()
    in_maps = make_in_maps(inputs["x"], inputs["Wq"], inputs["Wk"], inputs["Wv"])
    res = run_bass_kernel_spmd(nc, in_maps, list(range(NCORES)))
    return assemble(res.results)


# revision 15
# speedup vs baseline: 1.1234x; 1.0774x over previous
"""Causal self-attention (softmax over the QUERY axis) for Trainium2, 8 cores.

Reference semantics (B=2, S=2048, D=1024, H=16, HD=64):
    q = x @ Wq; k = x @ Wk; v = x @ Wv          (per batch)
    s[b,h,q,k] = <q_bqh, k_bkh>;  mask k > q -> -inf
    w = softmax(s / sqrt(1024), axis=q)          # normalize over QUERY axis
    ctx[b,q,h,:] = sum_k w[b,h,q,k] * v[b,k,h,:]

Sharding: core c handles batch b = c // 4 and head group g = c % 4
(4 heads: 4g..4g+3).  Per core everything is done in a transposed
score layout S^T[k, q], which makes the query-axis softmax a FREE-AXIS
reduction, and the 1/Z[k] normalizer folds into V rows (no per-element
divide): ctx[q,d] = sum_k exp(s)/Z[k] * v[k,d] = sum_k exp(s) * (v[k,d]/Z[k]).

Key structure (v2, ACT-engine-centric):
  - Causal diag masking is done IN PSUM via one extra matmul per score
    row: I128^T @ TRI adds -1e6 to the strictly-lower part of the 128x128
    diagonal block, so exp() produces exact zeros and the row sum (Z) is
    correct with no post-hoc correction (no gpsimd selects, no inv sums).
  - Score rows ping-pong between a 4-bank [128,2048] and a 3-bank
    [128,1536] PSUM tile, so each row is ONE activation instruction;
    short rows (kt>=8) are packed in pairs into one activation.
  - Z: accum_out on solo rows (kt 0..7), DVE post-zero row-reduce for
    packed rows (kt 8..15).
  - exp() is the only real work on the Scalar queue (input DMAs moved to
    gpsimd/vector queues); E is stored per head as one packed [128,17408]
    bf16 tile (row kt at col E_OFF[kt]).
  - A short burst of dummy matmuls during the input-DMA window pre-warms
    the PE HAM clock gate so real matmuls start at 2.4 GHz.

Device layouts (per core):
    xT  [1024, 2048] bf16 (host-transposed)  -> SBUF [128, 8, 2048]
    Wq/Wk/Wv column slices [1024, 256] bf16  -> SBUF [128, 8, 256]
    qT/kT  [128(2 heads x 64), 2 pairs, 2048] bf16 (projection output)
    v      [128(s in tile), 16 kt, 256(4 heads x 64)] bf16 (scaled by 1/Z in place)
    E      per head [128, 17408] bf16, row kt at cols [E_OFF[kt], +2048-128kt)
    out    [256(4 heads x 64), 2048] f32 = ctx^T; host transposes back.
"""

import numpy as np
import ml_dtypes
from contextlib import ExitStack

import concourse.bass as bass
import concourse.tile as tile
from concourse import bacc, mybir
from concourse.bass_utils import run_bass_kernel_spmd

BF16 = mybir.dt.bfloat16
F32 = mybir.dt.float32

B, S, D, H, HD = 2, 2048, 1024, 16, 64
NCORES = 8
HL = 4                       # heads per core
KC = D // 128                # 8 contraction chunks
KT = S // 128                # 16 key tiles
QC = S // 512                # 4 query chunks of 512
SCALE = 1.0 / float(np.sqrt(np.float32(D)))   # 1/32
MASK_BIG = -1.0e6

W_ROW = [S - 128 * kt for kt in range(KT)]          # valid width of E row kt
E_OFF = np.concatenate([[0], np.cumsum(W_ROW)]).astype(int)
E_TOT = int(E_OFF[-1])                              # 17408

# score-row emission plan per query chunk (descending heads 0..2).
# groups must fit a [128,1536] psum tile; rows 0-3 are emitted in two parts.
QC_GROUPS = {
    3: [(14, 15), (12, 13)],
    2: [(10, 11), (9,), (8,)],
    1: [(7,), (6,), (5,), (4,)],
    0: [(3,), (2,), (1,), (0,)],
}
# head 3 runs ascending in groups of 4 rows so pair-1 ctx can start early
G_GROUPS = {
    0: [(0,), (1,), (2,), (3,)],
    1: [(4,), (5,), (6,), (7,)],
    2: [(8,), (9,), (10, 11)],
    3: [(12, 13), (14, 15)],
}
# rows whose Z comes from a DVE row-sum instead of the ACT accumulator
DVE_Z = set(range(8, KT))


def _emit(ctx: ExitStack, tc: tile.TileContext, out_ap, xT, wq, wk, wv):
    nc = tc.nc
    Exp = mybir.ActivationFunctionType.Exp

    consts = ctx.enter_context(tc.tile_pool(name="consts", bufs=1))
    qkp = ctx.enter_context(tc.tile_pool(name="qk", bufs=1))
    vp = ctx.enter_context(tc.tile_pool(name="v", bufs=1))
    epool = ctx.enter_context(tc.tile_pool(name="e", bufs=3))
    zpool = ctx.enter_context(tc.tile_pool(name="z", bufs=4))
    outp = ctx.enter_context(tc.tile_pool(name="outp", bufs=1))
    # PSUM: two 3-bank score tiles (ping-pong) + two 1-bank proj/ctx tiles
    sc_ps = ctx.enter_context(tc.tile_pool(name="sc_ps", bufs=2, space="PSUM"))
    small_ps = ctx.enter_context(tc.tile_pool(name="small_ps", bufs=2, space="PSUM"))

    # ---- input DMAs: both HWDGE rings (sync + scalar).  The two scalar-ring
    # issues happen before any activation work exists, so the ACT queue is
    # free again well before the first exp ----
    xT_r = xT.rearrange("(c p) s -> p c s", p=128)
    xT_cs = [None] * 4

    def load_chunk(sc, eng):
        xT_cs[sc] = consts.tile([128, KC, 512], BF16, tag=f"xT{sc}",
                                name=f"xT{sc}_sb")
        eng.dma_start(out=xT_cs[sc], in_=xT_r[:, :, 512 * sc:512 * sc + 512])

    w_sb = {}

    def load_w(name, t):
        w_sb[name] = consts.tile([128, KC, HL * HD], BF16, tag=f"w{name}",
                                 name=f"w{name}_sb")
        nc.sync.dma_start(out=w_sb[name], in_=t.rearrange("(c p) n -> p c n", p=128))

    load_w("q", wq)
    load_w("k", wk)
    load_w("v", wv)
    for sc in (3, 2, 1, 0):
        load_chunk(sc, nc.scalar)

    # mask constants built on-device (gpsimd) -- no DMA descriptors needed.
    # ident = keep j==p of ones (two is_ge selects); tri = -1e6 where j < p.
    ident_sb = consts.tile([128, 128], BF16, tag="ident", name="ident_sb")
    tri_sb = consts.tile([128, 128], BF16, tag="tri", name="tri_sb")
    scr_sb = consts.tile([128, 128], BF16, tag="scr", name="scr_sb")
    nc.gpsimd.memset(scr_sb, 1.0)
    nc.gpsimd.affine_select(ident_sb, scr_sb, pattern=[[1, 128]],
                            compare_op=mybir.AluOpType.is_ge, fill=0.0,
                            base=0, channel_multiplier=-1)
    nc.gpsimd.affine_select(ident_sb, ident_sb, pattern=[[-1, 128]],
                            compare_op=mybir.AluOpType.is_ge, fill=0.0,
                            base=0, channel_multiplier=1)
    nc.gpsimd.memset(scr_sb, MASK_BIG)
    nc.gpsimd.affine_select(tri_sb, scr_sb, pattern=[[-1, 128]],
                            compare_op=mybir.AluOpType.is_ge, fill=0.0,
                            base=-1, channel_multiplier=1)

    def xT_slice(c, lo, w):
        sc, o = divmod(lo, 512)
        assert o + w <= 512
        return xT_cs[sc][:, c, o:o + w]

    qT_sb = qkp.tile([128, 2, S], BF16, tag="qT")
    kT_sb = qkp.tile([128, 2, S], BF16, tag="kT")
    v_sb = vp.tile([128, KT, HL * HD], BF16, tag="v")
    out_sb = outp.tile([128, 2, S], F32, tag="out")
    # ---- PE warm-up: dummy matmuls during the DMA window so HAM reaches
    # K=8/8 before the first projection chain (8 disjoint regions so no
    # write-after-write sync gets inserted between them) ----
    warm = consts.tile([128, 256], BF16, tag="warm", name="warm_sb")
    nc.vector.memset(warm, 0.0)
    wps = sc_ps.tile([128, 1536], F32, tag="sc", name="warmps")
    for i in range(30):
        r = 256 * (i % 6)
        nc.tensor.matmul(wps[:, r:r + 256], warm[:, 0:128], warm,
                         start=True, stop=True)

    def proj_chain(name, pair, qc):
        dst = qT_sb if name == "q" else kT_sb
        ps = small_ps.tile([128, 512], F32, tag="ps512", name="pps")
        for c in range(KC):
            nc.tensor.matmul(
                ps,
                w_sb[name][:, c, 128 * pair:128 * pair + 128],
                xT_cs[qc][:, c, :],
                start=(c == 0), stop=(c == KC - 1),
            )
        nc.vector.tensor_copy(dst[:, pair, 512 * qc:512 * qc + 512], ps)

    def proj_v(s0, s1):
        # v natural layout: out partitions = s-within-tile, cols = 4 heads x 64
        # (chain st only touches xT chunk st//4)
        for st in range(s0, s1):
            ps = small_ps.tile([128, HL * HD], F32, tag="ps512", name="pps")
            for c in range(KC):
                nc.tensor.matmul(
                    ps,
                    xT_slice(c, 128 * st, 128),
                    w_sb["v"][:, c, :],
                    start=(c == 0), stop=(c == KC - 1),
                )
            nc.vector.tensor_copy(v_sb[:, st, :], ps)

    def alloc_head(h):
        zp = zpool.tile([128, KT, 2], F32, tag="zp", name=f"zp{h}")
        nc.vector.memset(zp, 0.0)
        return {
            "h": h,
            "E": epool.tile([128, E_TOT], BF16, tag="E", name=f"E{h}"),
            "zp": zp,
            "zi": zpool.tile([128, KT], F32, tag="zi", name=f"zi{h}"),
        }

    def _mm_row(ps, st, kt, po, lo, w):
        """matmuls for score row kt cols [lo, lo+w) into ps at offset po,
        chunked at psum bank boundaries; mask MM when the diag is included."""
        h = st["h"]
        pair, half = divmod(h, 2)
        pb = 64 * half
        lhsT = kT_sb[pb:pb + 64, pair, 128 * kt:128 * kt + 128]
        q0 = 128 * kt + lo
        c = po
        first = True
        while c < po + w:
            c1 = min(po + w, (c // 512 + 1) * 512)
            nc.tensor.matmul(
                ps[:, c:c1], lhsT,
                qT_sb[pb:pb + 64, pair, q0 + c - po:q0 + c1 - po],
                start=True, stop=not (first and lo == 0),
            )
            first = False
            c = c1
        if lo == 0:
            # causal mask: -1e6 on the strictly-lower diag entries -> exp = 0
            nc.tensor.matmul(ps[:, po:po + 128], ident_sb, tri_sb,
                             start=False, stop=True)

    def score_group(st, rows):
        """One psum tile (<=1536 cols): a full short row, a 1536/rest part of
        a long row, or a packed pair of short rows.  One exp() per tile."""
        for kt in rows:
            assert W_ROW[kt] <= 1536 or len(rows) == 1
        if len(rows) == 1 and W_ROW[rows[0]] > 1536:
            kt = rows[0]
            W = W_ROW[kt]
            for part, (lo, w) in enumerate(((0, 1536), (1536, W - 1536))):
                ps = sc_ps.tile([128, 1536], F32, tag="sc", name="scps")
                _mm_row(ps, st, kt, 0, lo, w)
                nc.scalar.activation(
                    st["E"][:, int(E_OFF[kt]) + lo:int(E_OFF[kt]) + lo + w],
                    ps[:, 0:w], Exp, scale=SCALE,
                    accum_out=st["zp"][:, kt, part:part + 1])
            return
        ps = sc_ps.tile([128, 1536], F32, tag="sc", name="scps")
        po = 0
        offs = []
        for kt in rows:
            offs.append(po)
            _mm_row(ps, st, kt, po, 0, W_ROW[kt])
            po += W_ROW[kt]
        e0 = int(E_OFF[rows[0]])
        if len(rows) == 1 and rows[0] not in DVE_Z:
            nc.scalar.activation(st["E"][:, e0:e0 + po], ps[:, 0:po],
                                 Exp, scale=SCALE,
                                 accum_out=st["zp"][:, rows[0], 0:1])
        else:
            nc.scalar.activation(st["E"][:, e0:e0 + po], ps[:, 0:po],
                                 Exp, scale=SCALE)
            for kt in rows:
                nc.vector.tensor_reduce(
                    st["zp"][:, kt, 0:1],
                    st["E"][:, int(E_OFF[kt]):int(E_OFF[kt]) + W_ROW[kt]],
                    axis=mybir.AxisListType.X, op=mybir.AluOpType.add,
                )

    def v2_scale(st, k0, k1):
        """finalize 1/Z for rows [k0, k1) and scale this head's V cols."""
        h = st["h"]
        n = k1 - k0
        zs = zpool.tile([128, KT], F32, tag="zs", name=f"zs{h}")
        nc.vector.tensor_reduce(zs[:, k0:k1], st["zp"][:, k0:k1, :],
                                axis=mybir.AxisListType.X,
                                op=mybir.AluOpType.add)
        nc.vector.reciprocal(st["zi"][:, k0:k1], zs[:, k0:k1])
        zia = st["zi"][:, k0:k1]
        zi_bc = bass.AP(tensor=zia.tensor, offset=zia.offset,
                        ap=[zia.ap[0], zia.ap[1], [0, HD]])
        nc.vector.tensor_mul(
            v_sb[:, k0:k1, HD * h:HD * h + HD],
            v_sb[:, k0:k1, HD * h:HD * h + HD],
            zi_bc,
        )

    def ctx_pair(sta, stb, qc):
        """col-packed ctx chains for a whole pair (heads sta, stb) at qc."""
        pair = sta["h"] // 2
        ps = small_ps.tile([128, 512], F32, tag="ps512", name="cpp")
        n_kt = 4 * qc + 4
        for kt in range(n_kt):
            q0 = max(512 * qc, 128 * kt)
            w = 512 * qc + 512 - q0
            for half, st in ((0, sta), (1, stb)):
                h = st["h"]
                lo = int(E_OFF[kt]) + q0 - 128 * kt
                nc.tensor.matmul(
                    ps[64 * half:64 * half + 64, q0 - 512 * qc:512],
                    v_sb[:, kt, HD * h:HD * h + HD],
                    st["E"][:, lo:lo + w],
                    start=(kt == 0), stop=(kt == n_kt - 1),
                    tile_position=(0, 64 * half),
                    skip_group_check=True,
                )
        nc.vector.tensor_copy(out_sb[:, pair, 512 * qc:512 * qc + 512], ps)

    def out_dma(pair, qc):
        nc.sync.dma_start(
            out=out_ap[128 * pair:128 * pair + 128, 512 * qc:512 * qc + 512],
            in_=out_sb[:, pair, 512 * qc:512 * qc + 512],
        )

    # ---- emission (order = scheduling priority) ----
    # Pair 0 (heads 0,1) is processed per query chunk, zipped, so the exp
    # stream starts as soon as xT chunk 3 lands and never waits on later
    # chunks; pair-1 projections trail each chunk as PE filler.
    st0 = alloc_head(0)
    st1 = alloc_head(1)
    for qc in (3, 2, 1, 0):
        proj_chain("q", 0, qc)
        proj_chain("k", 0, qc)
        for rows in QC_GROUPS[qc]:
            score_group(st0, rows)
        for rows in QC_GROUPS[qc]:
            score_group(st1, rows)
        proj_chain("q", 1, qc)
        proj_chain("k", 1, qc)
    proj_v(0, 4)
    v2_scale(st0, 0, 4)
    v2_scale(st1, 0, 4)
    ctx_pair(st0, st1, 0)
    out_dma(0, 0)
    # Head 2, with the pair-0 context chains and V projection trailing each
    # chunk as PE filler (E0/E1 are fully consumed before head 3 needs the
    # E pool slot).
    st2 = alloc_head(2)
    for qc, (vs0, g) in zip((3, 2, 1, 0), ((4, 1), (8, 2), (12, 3), (None, None))):
        for rows in QC_GROUPS[qc]:
            score_group(st2, rows)
        if vs0 is not None:
            proj_v(vs0, vs0 + 4)
            v2_scale(st0, vs0, vs0 + 4)
            v2_scale(st1, vs0, vs0 + 4)
            ctx_pair(st0, st1, g)
            out_dma(0, g)
    v2_scale(st2, 0, KT)
    st3 = alloc_head(3)
    for g in range(4):                # head 3 ascending; pair-1 ctx follows
        for rows in G_GROUPS[g]:
            score_group(st3, rows)
        v2_scale(st3, 4 * g, 4 * g + 4)
        ctx_pair(st2, st3, g)
        out_dma(1, g)


_PROG = None


def _build_program():
    global _PROG
    if _PROG is not None:
        return _PROG
    nc = bacc.Bacc("TRN2", target_bir_lowering=False, debug=False,
                   num_devices=NCORES)
    xT = nc.dram_tensor("xT", [D, S], BF16, kind="ExternalInput").ap()
    wq = nc.dram_tensor("wq", [D, HL * HD], BF16, kind="ExternalInput").ap()
    wk = nc.dram_tensor("wk", [D, HL * HD], BF16, kind="ExternalInput").ap()
    wv = nc.dram_tensor("wv", [D, HL * HD], BF16, kind="ExternalInput").ap()
    out = nc.dram_tensor("out", [HL * HD, S], F32, kind="ExternalOutput").ap()
    with tile.TileContext(nc) as tc:
        with ExitStack() as stack:
            _emit(stack, tc, out, xT, wq, wk, wv)
    nc.compile()
    _PROG = nc
    return nc


def make_in_maps(x, Wq, Wk, Wv):
    bf = ml_dtypes.bfloat16
    in_maps = []
    for core in range(NCORES):
        b, g = divmod(core, NCORES // B)
        cols = slice(HL * HD * g, HL * HD * (g + 1))
        in_maps.append({
            "xT": np.ascontiguousarray(np.asarray(x[b]).T).astype(bf),
            "wq": np.ascontiguousarray(np.asarray(Wq)[:, cols]).astype(bf),
            "wk": np.ascontiguousarray(np.asarray(Wk)[:, cols]).astype(bf),
            "wv": np.ascontiguousarray(np.asarray(Wv)[:, cols]).astype(bf),
        })
    return in_maps


def assemble(results):
    out = np.empty((B, S, H * HD), np.float32)
    for core in range(NCORES):
        b, g = divmod(core, NCORES // B)
        out[b, :, HL * HD * g:HL * HD * (g + 1)] = results[core]["out"].T
    return out


def kernel(**inputs):
    nc = _build_program()
    in_maps = make_in_maps(inputs["x"], inputs["Wq"], inputs["Wk"], inputs["Wv"])
    res = run_bass_kernel_spmd(nc, in_maps, list(range(NCORES)))
    return assemble(res.results)


# revision 16
# speedup vs baseline: 1.3194x; 1.1744x over previous
"""Causal self-attention (softmax over the QUERY axis) for Trainium2, 8 cores.

Reference semantics (B=2, S=2048, D=1024, H=16, HD=64):
    q = x @ Wq; k = x @ Wk; v = x @ Wv          (per batch)
    s[b,h,q,k] = <q_bqh, k_bkh>;  mask k > q -> -inf
    w = softmax(s / sqrt(1024), axis=q)          # normalize over QUERY axis
    ctx[b,q,h,:] = sum_k w[b,h,q,k] * v[b,k,h,:]

Sharding: core c handles batch b = c // 4 and head group g = c % 4
(4 heads: 4g..4g+3).  Per core everything is done in a transposed
score layout S^T[k, q], which makes the query-axis softmax a FREE-AXIS
reduction, and the 1/Z[k] normalizer folds into V rows (no per-element
divide): ctx[q,d] = sum_k exp(s)/Z[k] * v[k,d] = sum_k exp(s) * (v[k,d]/Z[k]).

Device layouts (per core):
    xT  [1024, 2048] bf16 (host-transposed)  -> SBUF [128, 8, 2048]
    Wq/Wk/Wv column slices [1024, 256] bf16  -> SBUF [128, 8, 256]
    qT/kT  [128(2 heads x 64), 2 pairs, 2048] bf16 (projection output)
    v      [128(s in tile), 16 kt, 256(4 heads x 64)] bf16
    E      packed exp(scores^T): row kt occupies cols [off_kt, off_kt+2048-128kt)
    out    [256(4 heads x 64), 2048] f32 = ctx^T; host transposes back.
"""

import numpy as np
import ml_dtypes
from contextlib import ExitStack

import concourse.bass as bass
import concourse.tile as tile
from concourse import bacc, mybir
from concourse.bass_utils import run_bass_kernel_spmd

BF16 = mybir.dt.bfloat16
F32 = mybir.dt.float32

B, S, D, H, HD = 2, 2048, 1024, 16, 64
NCORES = 8
HL = 4                       # heads per core
KC = D // 128                # 8 contraction chunks
KT = S // 128                # 16 key tiles
QC = S // 512                # 4 query chunks of 512
SCALE = 1.0 / float(np.sqrt(np.float32(D)))   # 1/32

W_ROW = [S - 128 * kt for kt in range(KT)]          # valid width of E row kt
E_OFF = np.concatenate([[0], np.cumsum(W_ROW)]).astype(int)
E_TOT = int(E_OFF[-1])                              # 17408



def _emit(ctx: ExitStack, tc: tile.TileContext, out_ap, xT, wq, wk, wv):
    nc = tc.nc
    Exp = mybir.ActivationFunctionType.Exp

    consts = ctx.enter_context(tc.tile_pool(name="consts", bufs=1))
    qkp = ctx.enter_context(tc.tile_pool(name="qk", bufs=1))
    vp = ctx.enter_context(tc.tile_pool(name="v", bufs=1))
    epool = ctx.enter_context(tc.tile_pool(name="e", bufs=2))
    zpool = ctx.enter_context(tc.tile_pool(name="z", bufs=4))
    spool = ctx.enter_context(tc.tile_pool(name="scr", bufs=4))
    outp = ctx.enter_context(tc.tile_pool(name="outp", bufs=1))
    # scores rows: [128, 1536] = 3 banks x 2 bufs = 6 banks; projections and
    # ctx accumulations share one 2-slot [*, 512] pool (2 banks).
    sc_ps = ctx.enter_context(tc.tile_pool(name="sc_ps", bufs=2, space="PSUM"))
    small_ps = ctx.enter_context(tc.tile_pool(name="small_ps", bufs=2, space="PSUM"))

    # ---- loads: weights on the SP HWDGE ring, xT chunks on the ACT ring
    # (chunk 3 first: score rows are emitted descending) ----
    w_sb = {}
    for name, t in (("q", wq), ("k", wk), ("v", wv)):
        w_sb[name] = consts.tile([128, KC, HL * HD], BF16, tag=f"w{name}",
                                 name=f"w{name}_sb")
        nc.sync.dma_start(out=w_sb[name], in_=t.rearrange("(c p) n -> p c n", p=128))
    xT_r = xT.rearrange("(c p) s -> p c s", p=128)
    xT_cs = [None] * 4
    for sc in (3, 2, 1, 0):
        xT_cs[sc] = consts.tile([128, KC, 512], BF16, tag=f"xT{sc}",
                                name=f"xT{sc}_sb")
        nc.scalar.dma_start(out=xT_cs[sc],
                            in_=xT_r[:, :, 512 * sc:512 * sc + 512])

    def xT_slice(c, lo, w):
        sc, o = divmod(lo, 512)
        assert o + w <= 512
        return xT_cs[sc][:, c, o:o + w]

    warm = consts.tile([128, 256], BF16, tag="warm", name="warm_sb")
    nc.vector.memset(warm, 0.0)
    wps = sc_ps.tile([128, 1536], F32, tag="sc", name="warmps")
    for i in range(30):
        r = 256 * (i % 6)
        nc.tensor.matmul(wps[:, r:r + 256], warm[:, 0:128], warm,
                         start=True, stop=True)

    qT_sb = qkp.tile([128, 2, S], BF16, tag="qT")
    kT_sb = qkp.tile([128, 2, S], BF16, tag="kT")
    v_sb = vp.tile([128, KT, HL * HD], BF16, tag="v")
    v2_sb = vp.tile([128, KT, HL * HD], BF16, tag="v2")
    out_sb = outp.tile([128, 2, S], F32, tag="out")

    def proj_chain(name, pair, qc):
        dst = qT_sb if name == "q" else kT_sb
        ps = small_ps.tile([128, 512], F32, tag="ps512", name="pps")
        for c in range(KC):
            nc.tensor.matmul(
                ps,
                w_sb[name][:, c, 128 * pair:128 * pair + 128],
                xT_cs[qc][:, c, :],
                start=(c == 0), stop=(c == KC - 1),
            )
        nc.vector.tensor_copy(dst[:, pair, 512 * qc:512 * qc + 512], ps)

    def proj_v():
        # v natural layout: out partitions = s-within-tile, cols = 4 heads x 64
        for st in range(KT):
            ps = small_ps.tile([128, HL * HD], F32, tag="ps512", name="pps")
            for c in range(KC):
                nc.tensor.matmul(
                    ps,
                    xT_slice(c, 128 * st, 128),
                    w_sb["v"][:, c, :],
                    start=(c == 0), stop=(c == KC - 1),
                )
            nc.vector.tensor_copy(v_sb[:, st, :], ps)

    def alloc_head(h):
        zp = zpool.tile([128, KT, 2], F32, tag="zp", name=f"zp{h}")
        inv = zpool.tile([128, KT], F32, tag="inv", name=f"inv{h}")
        nc.vector.memset(zp, 0.0)
        nc.vector.memset(inv, 0.0)
        return {"zp": zp, "inv": inv, "e": [None] * KT, "h": h}

    def score_row(st, kt):
        """scores^T row kt for head st['h']: matmuls + exp(+Z accum) + diag fix."""
        h = st["h"]
        pair, half = divmod(h, 2)
        pb = 64 * half
        q0k = 128 * kt
        W = S - q0k
        # rows 4..15 get a third slot so the next pair's score rows never
        # wait on ctx chains releasing E (rows 0..3 are too big to afford
        # a third copy, but they are also the last ones the next head
        # reaches, by which point the ctx chains have freed them).
        e_row = epool.tile([128, W], BF16, tag=f"E{kt}", name=f"e{kt}",
                           bufs=(3 if kt >= 4 else 2))
        st["e"][kt] = e_row
        lhsT = kT_sb[pb:pb + 64, pair, q0k:q0k + 128]   # [64, 128]
        tiles = [(q0k, min(W, 1536))]
        if W > 1536:
            tiles.append((q0k + 1536, W - 1536))
        dve_z = kt >= 8    # short rows: Z via DVE post-zero sum (ACT stays hot)
        for ti, (lo, w) in enumerate(tiles):
            ps = sc_ps.tile([128, w], F32, tag="sc", name="scps")
            c0 = 0
            while c0 < w:
                c1 = min(w, c0 + 512)
                nc.tensor.matmul(
                    ps[:, c0:c1],
                    lhsT,
                    qT_sb[pb:pb + 64, pair, lo + c0:lo + c1],
                    start=True, stop=True,
                )
                c0 = c1
            if dve_z:
                nc.scalar.activation(
                    e_row[:, lo - q0k:lo - q0k + w], ps[:, 0:w],
                    Exp, scale=SCALE,
                )
            else:
                nc.scalar.activation(
                    e_row[:, lo - q0k:lo - q0k + w], ps[:, 0:w],
                    Exp, scale=SCALE,
                    accum_out=st["zp"][:, kt, ti:ti + 1],
                )
        # diagonal block: cols [0, 128) hold q in [128kt, 128kt+128);
        # entries with q < k (j < p) are invalid.
        diag = e_row[:, 0:128]
        if not dve_z:
            # gather the invalid part (its sum is subtracted from Z);
            # is_lt is unimplemented in walrus codegen, so use is_ge with
            # negated affine coefficients (j < p <=> p - j - 1 >= 0).
            scr = spool.tile([128, 128], BF16, tag="scr", name="scr")
            nc.gpsimd.affine_select(
                scr, diag, pattern=[[-1, 128]],
                compare_op=mybir.AluOpType.is_ge, fill=0.0,
                base=-1, channel_multiplier=1,
            )
            nc.vector.tensor_reduce(
                st["inv"][:, kt:kt + 1], scr,
                axis=mybir.AxisListType.X, op=mybir.AluOpType.add,
            )
        nc.gpsimd.affine_select(
            diag, diag, pattern=[[1, 128]],
            compare_op=mybir.AluOpType.is_ge, fill=0.0,
            base=0, channel_multiplier=-1,
        )
        if dve_z:
            # post-zero row sum is exactly the valid Z contribution
            nc.vector.tensor_reduce(
                st["zp"][:, kt, 0:1], e_row[:, 0:W],
                axis=mybir.AxisListType.X, op=mybir.AluOpType.add,
            )

    def z_v2(st, k0, k1):
        """finalize Z for rows [k0, k1) and scale V rows by 1/Z."""
        h = st["h"]
        n = k1 - k0
        zs = zpool.tile([128, n], F32, tag="zs", name="zs")
        nc.vector.tensor_reduce(zs, st["zp"][:, k0:k1, :],
                                axis=mybir.AxisListType.X,
                                op=mybir.AluOpType.add)
        zv = zpool.tile([128, n], F32, tag="zv", name="zv")
        nc.vector.tensor_sub(zv, zs, st["inv"][:, k0:k1])
        zi = zpool.tile([128, n], F32, tag="zi", name="zi")
        nc.vector.reciprocal(zi, zv)
        zia = zi[:, :]
        zi_bc = bass.AP(tensor=zia.tensor, offset=zia.offset,
                        ap=[zia.ap[0], zia.ap[1], [0, HD]])
        nc.vector.tensor_mul(
            v2_sb[:, k0:k1, HD * h:HD * h + HD],
            v_sb[:, k0:k1, HD * h:HD * h + HD],
            zi_bc,
        )

    def ctx_chain(st, qc):
        """one solo ctx^T accumulation chain for (head, qc) + copy to out_sb."""
        h = st["h"]
        pair, half = divmod(h, 2)
        ps = small_ps.tile([64, 512], F32, tag="ps512", name="cps")
        n_kt = 4 * qc + 4
        for kt in range(n_kt):
            q0 = max(512 * qc, 128 * kt)
            w = 512 * qc + 512 - q0
            rhs = st["e"][kt][:, q0 - 128 * kt:q0 - 128 * kt + w]
            nc.tensor.matmul(
                ps[:, q0 - 512 * qc:512],
                v2_sb[:, kt, HD * h:HD * h + HD],
                rhs,
                start=(kt == 0), stop=(kt == n_kt - 1),
            )
        nc.vector.tensor_copy(
            out_sb[64 * half:64 * half + 64, pair, 512 * qc:512 * qc + 512], ps)

    def out_dma(pair, qc):
        nc.sync.dma_start(
            out=out_ap[128 * pair:128 * pair + 128, 512 * qc:512 * qc + 512],
            in_=out_sb[:, pair, 512 * qc:512 * qc + 512],
        )

    def ctx_pair_packed(sta, stb, qc):
        """col-packed ctx chains for a whole pair (heads sta, stb) at qc."""
        pair = sta["h"] // 2
        ps = small_ps.tile([128, 512], F32, tag="ps512", name="cpp")
        n_kt = 4 * qc + 4
        for kt in range(n_kt):
            q0 = max(512 * qc, 128 * kt)
            w = 512 * qc + 512 - q0
            for half, st in ((0, sta), (1, stb)):
                h = st["h"]
                rhs = st["e"][kt][:, q0 - 128 * kt:q0 - 128 * kt + w]
                nc.tensor.matmul(
                    ps[64 * half:64 * half + 64, q0 - 512 * qc:512],
                    v2_sb[:, kt, HD * h:HD * h + HD],
                    rhs,
                    start=(kt == 0), stop=(kt == n_kt - 1),
                    tile_position=(0, 64 * half),
                    skip_group_check=True,
                )
        nc.vector.tensor_copy(out_sb[:, pair, 512 * qc:512 * qc + 512], ps)

    # ---- emission (order = scheduling priority; heads' score rows always
    # outrank filler work so head transitions have no priority bubble) ----
    st0 = alloc_head(0)
    for qc in (3, 2, 1, 0):           # head 0 interleaved with its projections
        proj_chain("q", 0, qc)
        proj_chain("k", 0, qc)
        for kt in range(4 * qc + 3, 4 * qc - 1, -1):
            score_row(st0, kt)
    st1 = alloc_head(1)
    for kt in range(KT - 1, -1, -1):  # head 1 rows outrank all filler
        score_row(st1, kt)
    proj_v()                          # filler during heads 0-1 exp waits
    z_v2(st0, 0, KT)                  # (after proj_v: v_sb RAW order)
    z_v2(st1, 0, KT)
    for qc in (3, 2, 1, 0):           # pair-1 projections: filler
        proj_chain("q", 1, qc)
        proj_chain("k", 1, qc)
    st2 = alloc_head(2)
    for kt in range(KT - 1, -1, -1):  # E slots: rows 4-15 have a 3rd slot;
        score_row(st2, kt)            # rows 0-3 wait on the chain below
    ctx_pair_packed(st0, st1, 0)      # frees pair-0's E rows 0-3 early
    out_dma(0, 0)
    z_v2(st2, 0, KT)
    # head 3: ascending rows, per-group Z; overlaps head 2 on ACT since its
    # E slots are already free (3rd slot / chain-0 release)
    st3 = alloc_head(3)
    for g in range(4):
        for kt in range(4 * g, 4 * g + 4):
            score_row(st3, kt)
        z_v2(st3, 4 * g, 4 * g + 4)
        if g >= 1:                    # rest of pair-0 ctx: fills PE slack
            ctx_pair_packed(st0, st1, g)
            out_dma(0, g)
    for g in range(4):                # pair-1 ctx: packed, progressive
        ctx_pair_packed(st2, st3, g)
        out_dma(1, g)


_PROG = None


def _build_program():
    global _PROG
    if _PROG is not None:
        return _PROG
    nc = bacc.Bacc("TRN2", target_bir_lowering=False, debug=False,
                   num_devices=NCORES)
    xT = nc.dram_tensor("xT", [D, S], BF16, kind="ExternalInput").ap()
    wq = nc.dram_tensor("wq", [D, HL * HD], BF16, kind="ExternalInput").ap()
    wk = nc.dram_tensor("wk", [D, HL * HD], BF16, kind="ExternalInput").ap()
    wv = nc.dram_tensor("wv", [D, HL * HD], BF16, kind="ExternalInput").ap()
    out = nc.dram_tensor("out", [HL * HD, S], F32, kind="ExternalOutput").ap()
    with tile.TileContext(nc) as tc:
        with ExitStack() as stack:
            _emit(stack, tc, out, xT, wq, wk, wv)
    nc.compile()
    _PROG = nc
    return nc


def make_in_maps(x, Wq, Wk, Wv):
    bf = ml_dtypes.bfloat16
    in_maps = []
    for core in range(NCORES):
        b, g = divmod(core, NCORES // B)
        cols = slice(HL * HD * g, HL * HD * (g + 1))
        in_maps.append({
            "xT": np.ascontiguousarray(np.asarray(x[b]).T).astype(bf),
            "wq": np.ascontiguousarray(np.asarray(Wq)[:, cols]).astype(bf),
            "wk": np.ascontiguousarray(np.asarray(Wk)[:, cols]).astype(bf),
            "wv": np.ascontiguousarray(np.asarray(Wv)[:, cols]).astype(bf),
        })
    return in_maps


def assemble(results):
    out = np.empty((B, S, H * HD), np.float32)
    for core in range(NCORES):
        b, g = divmod(core, NCORES // B)
        out[b, :, HL * HD * g:HL * HD * (g + 1)] = results[core]["out"].T
    return out


def kernel(**inputs):
    nc = _build_program()
    in_maps = make_in_maps(inputs["x"], inputs["Wq"], inputs["Wk"], inputs["Wv"])
    res = run_bass_kernel_spmd(nc, in_maps, list(range(NCORES)))
    return assemble(res.results)

